# revision 12
# baseline (speedup 1.0000x reference)
"""InternLM3 custom attention on 8 TRN2 NeuronCores.

Sharding: heads 4-per-core (qk_w/v_w column-parallel by head); AllToAll
converts the attention output from head-sharded to sequence-sharded;
o-projection runs sequence-parallel (full o_w per core) so each core
emits a [256, 2048] output slice.

All matmuls run in bf16 (fp32 PSUM accumulation). RoPE on X (queries =
raw hidden) is precomputed on the host; RoPE on K is applied per
projection chunk with offset-partition DVE ops (no swap DMAs).
Attention is computed transposed (S^T[k, q]) so softmax probabilities
feed the PV matmul as the moving operand; the denominator rides as a
ones column appended to V. Causality: strictly-upper k-chunks are
skipped, diagonal-band chunks compute only the live q-subrange.
Per (head, q-block): all score matmuls are emitted before all PV
matmuls so the Act engine (exp) stays saturated; projection of seq
chunk j+1 is emitted before attention block j to fill PE gaps.
"""

import sys

sys.path.insert(0, "/opt/trn_rl_repo")

import numpy as np
import ml_dtypes

import concourse.bass as bass
import concourse.tile as tile
from concourse import bacc, mybir
from concourse.bass import ds, ts
from concourse.bass_utils import run_bass_kernel_spmd

F32 = mybir.dt.float32
BF16 = mybir.dt.bfloat16
NCORES = 8
S = 2048          # sequence
HID = 2048        # hidden
NH = 32           # total heads
HD = 64           # head dim
HPC = NH // NCORES      # heads per core = 4
DPC = HPC * HD          # head-dims per core = 256
SSL = S // NCORES       # output seq slice per core = 256
VW = 68                 # interleaved V stride: 64 dims + 1 ones + 3 pad
ROPE_THETA = 10000.0


def build_program():
    nc = bacc.Bacc("TRN2", target_bir_lowering=False, debug=False,
                   num_devices=NCORES)

    # ---- I/O (all bf16 except the final output) ----
    hidT = nc.dram_tensor("hidT", [HID, S], BF16, kind="ExternalInput").ap()
    qkwT = nc.dram_tensor("qkwT", [HID, DPC], BF16, kind="ExternalInput").ap()
    vwT = nc.dram_tensor("vwT", [HID, DPC], BF16, kind="ExternalInput").ap()
    owT = nc.dram_tensor("owT", [HID, HID], BF16, kind="ExternalInput").ap()
    xT_in = nc.dram_tensor("xT", [DPC, S], BF16, kind="ExternalInput").ap()
    permM = nc.dram_tensor("permM", [128, 128], BF16, kind="ExternalInput").ap()
    cosT = nc.dram_tensor("cosT", [128, S], BF16, kind="ExternalInput").ap()
    sinT = nc.dram_tensor("sinT", [128, S], BF16, kind="ExternalInput").ap()
    maskT = nc.dram_tensor("maskT", [128, 896], BF16, kind="ExternalInput").ap()
    out_sl = nc.dram_tensor("out_slice", [SSL, HID], F32,
                            kind="ExternalOutput").ap()

    with tile.TileContext(nc) as tc:
        with (
            nc.allow_low_precision(reason="bf16 matmuls, fp32 psum accum"),
            tc.tile_pool(name="const", bufs=1) as const,
            tc.tile_pool(name="dram", bufs=1, space="DRAM") as dram,
        ):
            # ---- persistent SBUF residents ----
            qkw_t = const.tile([128, 16, DPC], BF16)
            vw_t = const.tile([128, 16, DPC], BF16)
            for q in range(4):
                nc.scalar.dma_start(
                    out=qkw_t[:, ds(4 * q, 4), :],
                    in_=qkwT[ds(512 * q, 512), :].rearrange("(n p) d -> p n d", p=128))
                nc.scalar.dma_start(
                    out=vw_t[:, ds(4 * q, 4), :],
                    in_=vwT[ds(512 * q, 512), :].rearrange("(n p) d -> p n d", p=128))
            cos_t = const.tile([128, S], BF16)
            nc.scalar.dma_start(out=cos_t[:], in_=cosT)
            sin_t = const.tile([128, S], BF16)
            nc.scalar.dma_start(out=sin_t[:], in_=sinT)
            mask_t = const.tile([128, 896], BF16)
            nc.scalar.dma_start(out=mask_t[:], in_=maskT)
            perm_t = const.tile([128, 128], BF16)
            nc.scalar.dma_start(out=perm_t[:], in_=permM)
            xt = const.tile([128, 2, S], BF16)        # roped X^T (host)
            nc.scalar.dma_start(
                out=xt[:], in_=xT_in.rearrange("(t p) s -> p t s", p=128))

            owt = const.tile([128, 16, HID], BF16)    # o_w^T, gpsimd-prefetched
            kt = const.tile([128, 2, S], BF16)        # roped K^T
            v_t = const.tile([128, 16, HPC, VW], BF16)  # V interleaved + ones
            att_t = const.tile([128, 2, S], BF16)     # attn^T assembled
            nc.gpsimd.memset(v_t[:, :, :, 64:65], 1.0)
            for g in range(2):
                nc.gpsimd.dma_start(
                    out=owt[:, ds(8 * g, 8), :],
                    in_=owT[ds(1024 * g, 1024), :].rearrange("(n p) d -> p n d", p=128))

            with (
                tc.tile_pool(name="hq", bufs=20) as hpool,
                tc.tile_pool(name="kr", bufs=2) as krp,
                tc.tile_pool(name="ksw", bufs=2) as kswp,
                tc.tile_pool(name="pt", bufs=5) as ptp,
                tc.tile_pool(name="rr", bufs=2) as rrp,
                tc.tile_pool(name="rb", bufs=2) as rbp,
                tc.tile_pool(name="psk", bufs=1, space="PSUM") as psk,
                tc.tile_pool(name="psv", bufs=2, space="PSUM") as psv,
                tc.tile_pool(name="psw", bufs=1, space="PSUM") as psw,
                tc.tile_pool(name="pss", bufs=2, space="PSUM") as pss,
                tc.tile_pool(name="pspv", bufs=1, space="PSUM") as pspv,
            ):
                def emit_A(sq, split_dma):
                    """Project K,V for seq chunk [512*sq, 512*sq+512)."""
                    pk = psk.tile([128, 2, 512], F32, tag='pk', name='pk')
                    hqs = []
                    for hc in range(16):
                        hq = hpool.tile([128, 512], BF16, tag='hq')
                        eng = nc.scalar if (split_dma and hc % 2) else nc.sync
                        eng.dma_start(out=hq[:],
                                      in_=hidT[ts(hc, 128), ts(sq, 512)])
                        hqs.append(hq)
                        for m in range(2):
                            nc.tensor.matmul(
                                pk[:, m, :],
                                (qkw_t[:, hc, ts(m, 128)]),
                                (hq[:]),
                                start=(hc == 0), stop=(hc == 15))
                    # K rope per 128-dim chunk: kt = pk*cos + swap32(pk)*sinS
                    # (swap32 via a PE permutation matmul; PSUM operands are
                    # exempt from the same-start-partition rule)
                    for m in range(2):
                        cs = ds(512 * sq, 512)
                        kraw = krp.tile([128, 512], BF16, tag='kraw')
                        nc.scalar.copy(out=kraw[:], in_=pk[:, m, :])
                        kswp_ps = psw.tile([128, 512], F32, tag='ksw', name='ksw')
                        nc.tensor.matmul(kswp_ps[:], (perm_t[:]), (kraw[:]),
                                         start=True, stop=True)
                        ksw = kswp.tile([128, 512], BF16, tag='ksw')
                        nc.vector.tensor_mul(out=ksw[:],
                                             in0=kswp_ps[:], in1=sin_t[:, cs])
                        nc.vector.tensor_mul(out=kt[:, m, cs],
                                             in0=kraw[:], in1=cos_t[:, cs])
                        nc.vector.tensor_add(out=kt[:, m, cs],
                                             in0=kt[:, m, cs], in1=ksw[:])
                    # V projection: one PSUM accumulation group per 128-seq
                    # block (groups must live in distinct banks)
                    for s4 in range(4):
                        pv = psv.tile([128, 4, 64], F32, tag='pv', name='pv')
                        for hc in range(16):
                            nc.tensor.matmul(
                                pv[:],
                                (hqs[hc][:, ts(s4, 128)]),
                                (vw_t[:, hc, :]),
                                start=(hc == 0), stop=(hc == 15))
                        nc.vector.tensor_copy(
                            out=v_t[:, 4 * sq + s4, :, 0:64],
                            in_=pv[:])

                def emit_B(j):
                    """Attention for q block [512*j, 512*j+512), all heads."""
                    q0 = 512 * j
                    nk = 4 * (j + 1)
                    for h in range(HPC):
                        hp = 64 * (h % 2)
                        htl = h // 2
                        pvp = pspv.tile([HD + 1, 512], F32, tag='pvp', name='pvp')
                        pts = []
                        for i in range(nk):
                            r = 128 * i - q0
                            sub = max(r, 0)
                            sp = pss.tile([128, 512], F32, tag='sp', name='sp')
                            nc.tensor.matmul(
                                sp[:, sub:512],
                                (kt[hp:hp + HD, htl, ts(i, 128)]),
                                (xt[hp:hp + HD, htl, ds(q0 + sub, 512 - sub)]),
                                start=True, stop=True)
                            pt = ptp.tile([128, 512], BF16, tag='pt')
                            nc.scalar.activation(
                                out=pt[:, sub:512], in_=sp[:, sub:512],
                                func=mybir.ActivationFunctionType.Exp,
                                scale=0.125)
                            if r >= 0:      # diagonal band: causal mask
                                nc.vector.tensor_mul(
                                    out=pt[:, sub:512], in0=pt[:, sub:512],
                                    in1=mask_t[:, ds(384, 512 - sub)])
                            pts.append((pt, sub))
                        for i, (pt, sub) in enumerate(pts):
                            nc.tensor.matmul(
                                pvp[:, sub:512],
                                (v_t[:, i, h, 0:HD + 1]),
                                (pt[:, sub:512]),
                                start=(i == 0), stop=(i == nk - 1))
                        # normalize by the ones-row denominator
                        rec = rrp.tile([1, 512], F32, tag='rec')
                        nc.vector.reciprocal(out=rec[:], in_=pvp[HD:HD + 1, :])
                        rb = rbp.tile([128, 512], F32, tag='rb')
                        nc.gpsimd.partition_broadcast(rb[:], rec[:])
                        nc.vector.tensor_mul(
                            out=att_t[hp:hp + HD, htl, ds(q0, 512)],
                            in0=pvp[0:HD, :], in1=rb[hp:hp + HD, :])

                emit_A(0, True)
                emit_A(1, True)
                emit_B(0)
                emit_A(2, False)
                emit_B(1)
                emit_A(3, False)
                emit_B(2)
                emit_B(3)

            # =========== AllToAll: head-sharded -> seq-sharded ===========
            a2a_in = dram.tile([NCORES, 2, 128, SSL], BF16)
            a2a_out = dram.tile([16, 128, SSL], BF16)
            for d in range(NCORES):
                for t in range(2):
                    nc.sync.dma_start(out=a2a_in[d, t, :, :],
                                      in_=att_t[:, t, ts(d, SSL)])
            nc.gpsimd.collective_compute(
                "AllToAll",
                mybir.AluOpType.bypass,
                replica_groups=[list(range(NCORES))],
                ins=[a2a_in[:].opt()],
                outs=[a2a_out[:].opt()],
            )

            # =========== o-projection (sequence-parallel) ===========
            with (
                tc.tile_pool(name="af", bufs=1) as afp,
                tc.tile_pool(name="ob", bufs=1) as obp,
                tc.tile_pool(name="pso", bufs=8, space="PSUM") as pso,
            ):
                afull = afp.tile([128, 16, SSL], BF16)
                nc.sync.dma_start(
                    out=afull[:], in_=a2a_out[:].rearrange("n p s -> p n s"))
                osb = obp.tile([128, 2, HID], F32)
                po = [[pso.tile([128, 512], F32, tag='po', name='po')
                       for t in range(2)] for ob in range(4)]
                for hc in range(16):
                    for ob in range(4):
                        for t in range(2):
                            nc.tensor.matmul(
                                po[ob][t][:],
                                (afull[:, hc, ts(t, 128)]),
                                (owt[:, hc, ds(512 * ob, 512)]),
                                start=(hc == 0), stop=(hc == 15))
                for ob in range(4):
                    for t in range(2):
                        nc.scalar.copy(out=osb[:, t, ts(ob, 512)],
                                       in_=po[ob][t][:])
                nc.sync.dma_start(out=out_sl[ts(0, 128), :], in_=osb[:, 0, :])
                nc.scalar.dma_start(out=out_sl[ts(1, 128), :], in_=osb[:, 1, :])

    nc.compile()
    return nc


_PROGRAM = None


def _host_inputs(hidden_states, qk_w, v_w, o_w, position_ids):
    bf16 = ml_dtypes.bfloat16
    hs = np.asarray(hidden_states, dtype=np.float32)[0]          # [S, HID]
    qk_w = np.asarray(qk_w, dtype=np.float32)
    v_w = np.asarray(v_w, dtype=np.float32)
    o_w = np.asarray(o_w, dtype=np.float32)
    pos = np.asarray(position_ids)[0].astype(np.float64)         # [S]

    hidT = np.ascontiguousarray(hs.T)                            # [HID, S] f32
    hidT_bf = hidT.astype(bf16)
    owT_bf = np.ascontiguousarray(o_w.T).astype(bf16)            # [HID, HID]

    inv_freq = 1.0 / (ROPE_THETA ** (np.arange(0, HD, 2, dtype=np.float64) / HD))
    freqs = pos[None, :] * inv_freq[:, None]                     # [32, S]
    emb = np.concatenate([freqs, freqs], axis=0)                 # [64, S]
    cos1 = np.cos(emb).astype(np.float32)
    sin1 = np.sin(emb).astype(np.float32)
    sin_signed = sin1.copy()
    sin_signed[:HD // 2] *= -1.0                                 # fold rotate sign
    cosT_bf = np.tile(cos1, (2, 1)).astype(bf16)                 # [128, S]
    sinT_bf = np.tile(sin_signed, (2, 1)).astype(bf16)
    cos4 = np.tile(cos1, (4, 1))                                 # [256, S] f32
    sin4 = np.tile(sin_signed, (4, 1))

    kl = np.arange(128)[:, None]
    u = np.arange(896)[None, :]
    maskT_bf = (u >= kl + 384).astype(bf16)                      # [128, 896]

    # rotate-half permutation as a matmul: out = permM.T @ k -> out[d] = k[p(d)]
    permM_bf = np.zeros((128, 128), dtype=bf16)
    for d in range(128):
        p = (d // 64) * 64 + ((d % 64) + 32) % 64
        permM_bf[p, d] = 1.0

    in_maps = []
    for c in range(NCORES):
        rows = slice(DPC * c, DPC * (c + 1))
        xT = hidT[rows]                                          # [256, S] f32
        xTs = np.empty_like(xT)                                  # rotate_half rows
        for h in range(HPC):
            b = HD * h
            xTs[b:b + 32] = xT[b + 32:b + 64]
            xTs[b + 32:b + 64] = xT[b:b + 32]
        xTr = (xT * cos4 + xTs * sin4).astype(bf16)              # roped X^T
        in_maps.append({
            "hidT": hidT_bf,
            "qkwT": np.ascontiguousarray(qk_w[rows].T).astype(bf16),
            "vwT": np.ascontiguousarray(v_w[rows].T).astype(bf16),
            "owT": owT_bf,
            "xT": xTr,
            "permM": permM_bf,
            "cosT": cosT_bf,
            "sinT": sinT_bf,
            "maskT": maskT_bf,
        })
    return in_maps


def kernel(hidden_states, qk_w, v_w, o_w, position_ids, **extra):
    global _PROGRAM
    if _PROGRAM is None:
        _PROGRAM = build_program()
    in_maps = _host_inputs(hidden_states, qk_w, v_w, o_w, position_ids)
    res = run_bass_kernel_spmd(_PROGRAM, in_maps, list(range(NCORES)))
    out = np.concatenate([res.results[c]["out_slice"]
                          for c in range(NCORES)], axis=0)
    return out.reshape(1, S, HID).astype(np.float32)
